# revision 33
# baseline (speedup 1.0000x reference)
"""Multi-head attention forward (B=2, N=2048, C=1024, H=16) on 8 TRN2 NeuronCores.

Tensor-parallel over heads: core c owns heads {2c, 2c+1}. Each core computes
QKV projection for its heads, full attention for its 4 (batch, head)
instances, and a partial output projection against its 128 rows of w_proj.
The host sums the 8 partial projections and adds the bias (row-parallel TP;
the all-reduce is the host-side unshard).

Per-core layouts (all matmul inputs bf16, PSUM accumulation f32):
  xT    [1024, 4096]  x^T, channel-major (replicated)
  wqk   [1024, 256]   [Wq_h0|Wq_h1|Wk_h0|Wk_h1] columns, Wq pre-scaled by D^-0.5
  wv    [1024, 128]   [Wv_h0|Wv_h1]
  wproj [128, 1024]   rows 128c:128c+128 of w_proj
  out   [4096, 1024]  f32 partial projection output

Attention per (b, h): S^T tiles [k=128, q=512] = KT_chunk.T @ QT (d-major, no
transposes), exp on ACT directly from PSUM (scale D^-0.5 pre-folded into Wq,
no max-subtraction -- scores are N(0,1) bounded), then O^T accumulation
po[*,q] += [V|1|0pad].T @ exp(S^T): one N=512 matmul per key tile with the
128-wide padded V stationary; row 64 of po accumulates the softmax
denominator l for free. Normalize via fast-reciprocal + gpsimd
partition-broadcast + one DVE multiply; head-1 rows repacked to partitions
64:128 by an SBUF->SBUF DMA so the projection is a single K=128 GEMM.

Scheduling: all independent PE work (batch-1 stage 1, projections) is woven
into the attention loops' ACT-wait slots so the PE instruction stream never
idles -- sub-microsecond idle gaps repeated per iteration make the HAM
re-throttle the PE clock from 2.4 to 1.2 GHz. Exp->O matmuls are skewed two
iterations. DMA descriptor issue (~0.64us each per sequencer) is spread
across the sync/scalar/gpsimd queues, with the 16 first-matmul-critical
descriptors issued first.
"""

import numpy as np
import ml_dtypes

import concourse.bass as bass
import concourse.tile as tile
from concourse import bacc, mybir
from concourse.bass_utils import run_bass_kernel_spmd
from concourse.masks import make_identity

B, N, C = 2, 2048, 1024
H = 16
D = C // H          # 64
SCALE = D ** -0.5
NCORES = 8
T = B * N           # 4096 tokens
KT = C // 128       # 8 k-tiles over the C contraction
TOK_TILES = T // 128  # 32
NK = N // 128       # 16 key tiles per sequence
QB = 512            # q block width
NQB = N // QB       # 4
BF = mybir.dt.bfloat16
F32 = mybir.dt.float32

_NC_CACHE = {}


def build():
    nc = bacc.Bacc("TRN2", target_bir_lowering=False, debug=False,
                   num_devices=NCORES)
    xT = nc.dram_tensor("xT", [C, T], BF, kind="ExternalInput").ap()
    wqk = nc.dram_tensor("wqk", [C, 256], BF, kind="ExternalInput").ap()
    wv = nc.dram_tensor("wv", [C, 128], BF, kind="ExternalInput").ap()
    wproj = nc.dram_tensor("wproj", [128, C], BF, kind="ExternalInput").ap()
    out = nc.dram_tensor("out", [T, C], F32, kind="ExternalOutput").ap()

    with tile.TileContext(nc) as tc:
        with tc.tile_pool(name="const", bufs=1) as const, \
             tc.tile_pool(name="work", bufs=4) as work, \
             tc.tile_pool(name="ps", bufs=2, space="PSUM") as ps, \
             tc.tile_pool(name="pso", bufs=1, space="PSUM") as pso, \
             tc.tile_pool(name="psm", bufs=3, space="PSUM") as psm:

            xt_sb = const.tile([128, KT, T], BF, tag="xt")
            wqk_sb = const.tile([128, KT, 256], BF, tag="wqk")
            wv_sb = const.tile([128, KT, 128], BF, tag="wv")
            wproj_sb = const.tile([128, C], BF, tag="wproj")
            qk_sb = const.tile([128, 2, T], BF, tag="qk")   # [qchan|kchan, token]
            vt_sb = const.tile([128, T], BF, tag="vt")      # V^T [vchan, token]
            v_sb = const.tile([128, TOK_TILES, 2, 128], BF, tag="v")  # per head [V|1|0pad]
            # normalized O^T [dchan, token]; h0 rows 0:64 written by DVE,
            # h1 rows 64:128 filled by SBUF->SBUF DMA repack from ot1_sb
            otp_sb = const.tile([128, T], BF, tag="otp")
            ot1_sb = const.tile([64, T], BF, tag="ot1")
            ident = const.tile([128, 128], BF, tag="ident")

            make_identity(nc, ident[:])

            # Spread DMA descriptor issue across engine queues -- a single
            # sequencer issues one descriptor per ~0.64us, which would gate
            # startup. wqk on sync (first QK needs it), wv on vector,
            # wproj on gpsimd; x^T chunks alternate sync/vector, nt-major
            # so stage-1 can start after the first 8 chunks.
            # The first QK unit needs all 8 wqk chunks + the 8 nt0 x^T
            # chunks -- issue those 16 first, split across the two HWDGE
            # sequencers, before anything else.
            def xt_dma(eng, nt, kt):
                eng.dma_start(
                    out=xt_sb[:, kt, nt * 512:(nt + 1) * 512],
                    in_=xT[kt * 128:(kt + 1) * 128,
                           nt * 512:(nt + 1) * 512])

            for kt in range(KT):
                eng = nc.sync if kt % 2 == 0 else nc.scalar
                eng.dma_start(out=wqk_sb[:, kt, :],
                              in_=wqk[kt * 128:(kt + 1) * 128, :])
                xt_dma(nc.sync if kt % 2 == 1 else nc.scalar, 0, kt)
            for kt in range(KT):
                xt_dma(nc.sync if kt % 2 == 0 else nc.scalar, 1, kt)
                nc.gpsimd.dma_start(out=wv_sb[:, kt, :],
                                    in_=wv[kt * 128:(kt + 1) * 128, :])
            nc.gpsimd.dma_start(out=wproj_sb[:], in_=wproj[:, :])
            for nt in range(2, T // 512):
                for kt in range(KT):
                    xt_dma(nc.sync if kt % 2 == 0 else nc.gpsimd, nt, kt)

            nc.vector.memset(v_sb[:, :, :, 65:128], 0.0)
            nc.vector.memset(v_sb[:, :, :, 64:65], 1.0)

            # -- work units -------------------------------------------------
            def emit_qk(mt, nt):
                pmm = psm.tile([128, 512], F32, tag="mm")
                for kt in range(KT):
                    nc.tensor.matmul(
                        pmm[:],
                        wqk_sb[:, kt, mt * 128:(mt + 1) * 128],
                        xt_sb[:, kt, nt * 512:(nt + 1) * 512],
                        start=(kt == 0), stop=(kt == KT - 1))
                nc.vector.tensor_copy(
                    qk_sb[:, mt, nt * 512:(nt + 1) * 512], pmm[:])

            def emit_vt(nt):
                # V^T chunk [128 vchan, 512 tok] with wv stationary
                pv = psm.tile([128, 512], F32, tag="mm")
                for kt in range(KT):
                    nc.tensor.matmul(
                        pv[:],
                        wv_sb[:, kt, :],
                        xt_sb[:, kt, nt * 512:(nt + 1) * 512],
                        start=(kt == 0), stop=(kt == KT - 1))
                nc.vector.tensor_copy(
                    vt_sb[:, nt * 512:(nt + 1) * 512], pv[:])

            def emit_vtr(t):
                # PE-transpose V^T tile back to token-major [128 tok, 128 d]
                pt = psm.tile([128, 128], BF, tag="mm")
                nc.tensor.transpose(
                    pt[:], vt_sb[:, t * 128:(t + 1) * 128], ident[:])
                nc.vector.tensor_copy(v_sb[:, t, 0, 0:64], pt[:, 0:64])
                nc.vector.tensor_copy(v_sb[:, t, 1, 0:64], pt[:, 64:128])

            def emit_proj(g, use_scalar=False):
                # out_tile = O^T_packed.T @ wproj (K=128, both heads)
                ob = work.tile([128, C], F32, tag="outstage")
                for ntile in range(2):
                    pmm = psm.tile([128, 512], F32, tag="mm")
                    nc.tensor.matmul(
                        pmm[:],
                        otp_sb[:, g * 128:(g + 1) * 128],
                        wproj_sb[:, ntile * 512:(ntile + 1) * 512],
                        start=True, stop=True)
                    if use_scalar and ntile == 1:
                        nc.scalar.copy(ob[:, 512:1024], pmm[:])
                    else:
                        nc.vector.tensor_copy(
                            ob[:, ntile * 512:(ntile + 1) * 512], pmm[:])
                nc.sync.dma_start(
                    out=out[g * 128:(g + 1) * 128, :], in_=ob[:])

            def o_mms(po, es, b, kp, h, first, last):
                # po[65, 512] += [V|1]_kt.T @ exp(S^T)_kt for both k-tiles
                for j in range(2):
                    vt = b * NK + kp * 2 + j
                    nc.tensor.matmul(
                        po[:],
                        v_sb[:, vt, h, :],
                        es[:, j * 512:(j + 1) * 512],
                        start=(first and j == 0),
                        stop=(last and j == 1))

            # One attention q-block: S^T tiles -> exp -> O^T accumulation.
            # extras[kp] are unrelated PE work units woven into the ACT-wait
            # slots so the PE never idles (HAM stays at full clock).
            def emit_s2(b, h, qb, pre=(), extras=None):
                for u in pre:
                    u()
                hp = slice(h * 64, (h + 1) * 64)
                q0 = b * N + qb * QB
                po = pso.tile([128, 512], F32, tag="o")
                pending = []   # (es, kp), O-mms lag 2 iterations behind exp
                for kp in range(NK // 2):
                    pst = ps.tile([128, 1024], F32, tag="s")
                    for j in range(2):
                        k0 = b * N + (kp * 2 + j) * 128
                        nc.tensor.matmul(
                            pst[:, j * 512:(j + 1) * 512],
                            qk_sb[hp, 1, k0:k0 + 128],
                            qk_sb[hp, 0, q0:q0 + QB],
                            start=True, stop=True)
                    es = work.tile([128, 1024], BF, tag="es")
                    nc.scalar.activation(
                        es[:], pst[:], mybir.ActivationFunctionType.Exp)
                    if len(pending) >= 2:
                        pes, pkp = pending.pop(0)
                        o_mms(po, pes, b, pkp, h,
                              first=(pkp == 0), last=False)
                    if extras:
                        for u in extras.get(kp, ()):
                            u()
                    pending.append((es, kp))
                for pes, pkp in pending:
                    o_mms(po, pes, b, pkp, h,
                          first=(pkp == 0), last=(pkp == NK // 2 - 1))
                # normalize O^T rows by 1/l: fast reciprocal of row 64,
                # partition-broadcast, one elementwise multiply
                lrow = work.tile([1, 512], F32, tag="lrow")
                nc.vector.tensor_copy(lrow[:], po[64:65, :])
                linv = work.tile([1, 512], F32, tag="linv")
                nc.vector.reciprocal_approx_fast(linv[:], lrow[:])
                lb = work.tile([64, 512], F32, tag="lb")
                nc.gpsimd.partition_broadcast(lb[:], linv[:])
                if h == 0:
                    nc.vector.tensor_mul(
                        otp_sb[0:64, q0:q0 + QB], po[0:64, :], lb[:])
                else:
                    nc.vector.tensor_mul(
                        ot1_sb[:, q0:q0 + QB], po[0:64, :], lb[:])
                    # repack h1 rows into partitions 64:128 of the packed
                    # O^T tile (cross-partition SBUF->SBUF DMA, gpsimd
                    # queue to keep the sync sequencer free for out-DMAs)
                    nc.gpsimd.dma_start(
                        out=otp_sb[64:128, q0:q0 + QB],
                        in_=ot1_sb[:, q0:q0 + QB])

            # -- schedule ---------------------------------------------------
            def U(f, *a):
                return lambda: f(*a)

            def drain(q, kps=range(NK // 2), per=1):
                ex = {}
                for kp in kps:
                    ex[kp] = [q.popleft() for _ in range(min(per, len(q)))]
                return ex

            from collections import deque

            # minimal prefix for (b0, h0, qb0)
            emit_qk(1, 0)
            emit_qk(0, 0)
            emit_vt(0)
            emit_vtr(0)
            emit_vtr(1)
            # remaining batch-0 stage-1 units woven into qb0 just ahead of
            # their consumers (S needs KT chunk kp//2, O(kp) needs v tiles
            # 2kp..2kp+1 one iteration later)
            ex0 = {0: [U(emit_vtr, 2), U(emit_vtr, 3)],
                   1: [U(emit_qk, 1, 1), U(emit_vt, 1),
                       U(emit_vtr, 4), U(emit_vtr, 5)],
                   2: [U(emit_vtr, 6), U(emit_vtr, 7)],
                   3: [U(emit_qk, 1, 2), U(emit_vt, 2),
                       U(emit_vtr, 8), U(emit_vtr, 9)],
                   4: [U(emit_vtr, 10), U(emit_vtr, 11)],
                   5: [U(emit_qk, 1, 3), U(emit_vt, 3),
                       U(emit_vtr, 12), U(emit_vtr, 13)],
                   6: [U(emit_vtr, 14), U(emit_vtr, 15)]}
            emit_s2(0, 0, 0, extras=ex0)

            # batch-1 stage-1 spread across the rest of batch 0's attention
            qA = deque()
            for nt in range(4, 8):
                qA.append(U(emit_qk, 1, nt))
                qA.append(U(emit_qk, 0, nt))
                qA.append(U(emit_vt, nt))
                for t in range(4 * nt, 4 * nt + 4):
                    qA.append(U(emit_vtr, t))
            emit_s2(0, 0, 1, pre=[U(emit_qk, 0, 1)],
                    extras=drain(qA, kps=(0, 2, 4, 6)))
            emit_s2(0, 0, 2, pre=[U(emit_qk, 0, 2)],
                    extras=drain(qA, kps=(0, 2, 4, 6)))
            emit_s2(0, 0, 3, pre=[U(emit_qk, 0, 3)],
                    extras=drain(qA, kps=(0, 2, 4, 6)))
            for qb in range(NQB):
                emit_s2(0, 1, qb, extras=drain(qA, kps=(0, 2, 4, 6)))
            rest = list(qA)
            qA.clear()

            # batch-0 projection spread thinly across ALL of batch 1 so even
            # the final q-blocks keep PE filler work during ACT waits
            qB = deque(U(emit_proj, g) for g in range(NK))
            emit_s2(1, 0, 0, pre=rest, extras=drain(qB, kps=(1, 5)))
            for qb in range(1, NQB):
                emit_s2(1, 0, qb, extras=drain(qB, kps=(1, 5)))
            # batch-1 projection for q-blocks already finished weaves into
            # (b1, h1)'s later q-blocks; only the last q-block's tiles remain
            # for the tail
            emit_s2(1, 1, 0, extras=drain(qB, kps=(1, 5)))
            qC = deque()
            for qb in range(1, NQB):
                qC.extend(U(emit_proj, NK + (qb - 1) * 4 + i) for i in range(4))
                ex = drain(qB, kps=(1, 5))
                for kp, us in drain(qC, kps=(3, 7), per=2).items():
                    ex.setdefault(kp, []).extend(us)
                emit_s2(1, 1, qb, extras=ex)
            for u in qB:
                u()
            for u in qC:
                u()
            for i, g in enumerate(range(2 * NK - 4, 2 * NK)):
                emit_proj(g, use_scalar=(i % 2 == 0))
    nc.compile()
    return nc


def make_in_maps(x, w_qkv, w_proj):
    bf = ml_dtypes.bfloat16
    x2 = x.reshape(T, C)
    xT_np = np.ascontiguousarray(x2.T).astype(bf)
    in_maps = []
    for c in range(NCORES):
        s = c * 128
        wq = w_qkv[:, s:s + 128] * SCALE
        wk = w_qkv[:, C + s:C + s + 128]
        wqk_np = np.ascontiguousarray(
            np.concatenate([wq, wk], axis=1)).astype(bf)
        wv_np = np.ascontiguousarray(
            w_qkv[:, 2 * C + s:2 * C + s + 128]).astype(bf)
        wproj_np = np.ascontiguousarray(w_proj[s:s + 128, :]).astype(bf)
        in_maps.append({"xT": xT_np, "wqk": wqk_np, "wv": wv_np,
                        "wproj": wproj_np})
    return in_maps


def kernel(x, w_qkv, w_proj, b_proj):
    x = np.asarray(x, dtype=np.float32)
    w_qkv = np.asarray(w_qkv, dtype=np.float32)
    w_proj = np.asarray(w_proj, dtype=np.float32)
    b_proj = np.asarray(b_proj, dtype=np.float32)

    if "nc" not in _NC_CACHE:
        _NC_CACHE["nc"] = build()
    nc = _NC_CACHE["nc"]

    in_maps = make_in_maps(x, w_qkv, w_proj)
    res = run_bass_kernel_spmd(nc, in_maps, list(range(NCORES)))
    acc = np.zeros((T, C), dtype=np.float32)
    for r in res.results:
        acc += r["out"]
    acc += b_proj[None, :]
    return acc.reshape(B, N, C)


# revision 34
# speedup vs baseline: 1.0579x; 1.0579x over previous
"""Multi-head attention forward (B=2, N=2048, C=1024, H=16) on 8 TRN2 NeuronCores.

Tensor-parallel over heads: core c owns heads {2c, 2c+1}. Each core computes
QKV projection for its heads, full attention for its 4 (batch, head)
instances, and a partial output projection against its 128 rows of w_proj.
The host sums the 8 partial projections and adds the bias (row-parallel TP;
the all-reduce is the host-side unshard).

Per-core layouts (all matmul inputs bf16, PSUM accumulation f32):
  xT    [1024, 4096]  x^T, channel-major (replicated)
  wqk   [1024, 256]   [Wq_h0|Wq_h1|Wk_h0|Wk_h1] columns, Wq pre-scaled by D^-0.5
  wv    [1024, 128]   [Wv_h0|Wv_h1]
  wproj [128, 1024]   rows 128c:128c+128 of w_proj
  out   [4096, 1024]  f32 partial projection output

Attention per (b, h): S^T tiles [k=128, q=512] = KT_chunk.T @ QT (d-major, no
transposes), exp on ACT directly from PSUM (scale D^-0.5 pre-folded into Wq,
no max-subtraction -- scores are N(0,1) bounded), then O^T accumulation
po[*,q] += [V|1|0pad].T @ exp(S^T): one N=512 matmul per key tile with the
128-wide padded V stationary; row 64 of po accumulates the softmax
denominator l for free. Normalize via fast-reciprocal + gpsimd
partition-broadcast + one DVE multiply; head-1 rows repacked to partitions
64:128 by an SBUF->SBUF DMA so the projection is a single K=128 GEMM.

Scheduling: all independent PE work (batch-1 stage 1, projections) is woven
into the attention loops' ACT-wait slots so the PE instruction stream never
idles -- sub-microsecond idle gaps repeated per iteration make the HAM
re-throttle the PE clock from 2.4 to 1.2 GHz. Exp->O matmuls are skewed two
iterations. DMA descriptor issue (~0.64us each per sequencer) is spread
across the sync/scalar/gpsimd queues, with the 16 first-matmul-critical
descriptors issued first.
"""

import numpy as np
import ml_dtypes

import concourse.bass as bass
import concourse.tile as tile
from concourse import bacc, mybir
from concourse.bass_utils import run_bass_kernel_spmd
from concourse.masks import make_identity

B, N, C = 2, 2048, 1024
H = 16
D = C // H          # 64
SCALE = D ** -0.5
NCORES = 8
T = B * N           # 4096 tokens
KT = C // 128       # 8 k-tiles over the C contraction
TOK_TILES = T // 128  # 32
NK = N // 128       # 16 key tiles per sequence
QB = 512            # q block width
NQB = N // QB       # 4
BF = mybir.dt.bfloat16
F32 = mybir.dt.float32

_NC_CACHE = {}


def build():
    nc = bacc.Bacc("TRN2", target_bir_lowering=False, debug=False,
                   num_devices=NCORES)
    xT = nc.dram_tensor("xT", [C, T], BF, kind="ExternalInput").ap()
    wqk = nc.dram_tensor("wqk", [C, 256], BF, kind="ExternalInput").ap()
    wv = nc.dram_tensor("wv", [C, 128], BF, kind="ExternalInput").ap()
    wproj = nc.dram_tensor("wproj", [128, C], BF, kind="ExternalInput").ap()
    out = nc.dram_tensor("out", [T, C], F32, kind="ExternalOutput").ap()

    with tile.TileContext(nc) as tc:
        with tc.tile_pool(name="const", bufs=1) as const, \
             tc.tile_pool(name="work", bufs=4) as work, \
             tc.tile_pool(name="ps", bufs=2, space="PSUM") as ps:

            xt_sb = const.tile([128, KT, T], BF, tag="xt")
            wqk_sb = const.tile([128, KT, 256], BF, tag="wqk")
            wv_sb = const.tile([128, KT, 128], BF, tag="wv")
            wproj_sb = const.tile([128, C], BF, tag="wproj")
            qk_sb = const.tile([128, 2, T], BF, tag="qk")   # [qchan|kchan, token]
            vt_sb = const.tile([128, T], BF, tag="vt")      # V^T [vchan, token]
            v_sb = const.tile([128, TOK_TILES, 2, 128], BF, tag="v")  # per head [V|1|0pad]
            # normalized O^T [dchan, token]; h0 rows 0:64 written by DVE,
            # h1 rows 64:128 filled by SBUF->SBUF DMA repack from ot1_sb
            otp_sb = const.tile([128, T], BF, tag="otp")
            ot1_sb = const.tile([64, T], BF, tag="ot1")
            ident = const.tile([128, 128], BF, tag="ident")

            make_identity(nc, ident[:])

            # Spread DMA descriptor issue across engine queues -- a single
            # sequencer issues one descriptor per ~0.64us, which would gate
            # startup. wqk on sync (first QK needs it), wv on vector,
            # wproj on gpsimd; x^T chunks alternate sync/vector, nt-major
            # so stage-1 can start after the first 8 chunks.
            # The first QK unit needs all 8 wqk chunks + the 8 nt0 x^T
            # chunks -- issue those 16 first, split across the two HWDGE
            # sequencers, before anything else.
            def xt_dma(eng, nt, kt):
                eng.dma_start(
                    out=xt_sb[:, kt, nt * 512:(nt + 1) * 512],
                    in_=xT[kt * 128:(kt + 1) * 128,
                           nt * 512:(nt + 1) * 512])

            for kt in range(KT):
                eng = nc.sync if kt % 2 == 0 else nc.scalar
                eng.dma_start(out=wqk_sb[:, kt, :],
                              in_=wqk[kt * 128:(kt + 1) * 128, :])
                xt_dma(nc.sync if kt % 2 == 1 else nc.scalar, 0, kt)
            for kt in range(KT):
                xt_dma(nc.sync if kt % 2 == 0 else nc.scalar, 1, kt)
                nc.gpsimd.dma_start(out=wv_sb[:, kt, :],
                                    in_=wv[kt * 128:(kt + 1) * 128, :])
            nc.gpsimd.dma_start(out=wproj_sb[:], in_=wproj[:, :])
            for nt in range(2, T // 512):
                for kt in range(KT):
                    xt_dma(nc.sync if kt % 2 == 0 else nc.gpsimd, nt, kt)

            nc.vector.memset(v_sb[:, :, :, 65:128], 0.0)
            nc.vector.memset(v_sb[:, :, :, 64:65], 1.0)

            # -- work units -------------------------------------------------
            def emit_qk(mt, nt):
                pmm = ps.tile([128, 512], F32, tag="mm")
                for kt in range(KT):
                    nc.tensor.matmul(
                        pmm[:],
                        wqk_sb[:, kt, mt * 128:(mt + 1) * 128],
                        xt_sb[:, kt, nt * 512:(nt + 1) * 512],
                        start=(kt == 0), stop=(kt == KT - 1))
                nc.vector.tensor_copy(
                    qk_sb[:, mt, nt * 512:(nt + 1) * 512], pmm[:])

            def emit_vt(nt):
                # V^T chunk [128 vchan, 512 tok] with wv stationary
                pv = ps.tile([128, 512], F32, tag="mm")
                for kt in range(KT):
                    nc.tensor.matmul(
                        pv[:],
                        wv_sb[:, kt, :],
                        xt_sb[:, kt, nt * 512:(nt + 1) * 512],
                        start=(kt == 0), stop=(kt == KT - 1))
                nc.vector.tensor_copy(
                    vt_sb[:, nt * 512:(nt + 1) * 512], pv[:])

            def emit_vtr(t):
                # PE-transpose V^T tile back to token-major [128 tok, 128 d]
                pt = ps.tile([128, 128], BF, tag="mm")
                nc.tensor.transpose(
                    pt[:], vt_sb[:, t * 128:(t + 1) * 128], ident[:])
                nc.vector.tensor_copy(v_sb[:, t, 0, 0:64], pt[:, 0:64])
                nc.vector.tensor_copy(v_sb[:, t, 1, 0:64], pt[:, 64:128])

            def emit_proj(g, use_scalar=False):
                # out_tile = O^T_packed.T @ wproj (K=128, both heads)
                ob = work.tile([128, C], F32, tag="outstage")
                for ntile in range(2):
                    pmm = ps.tile([128, 512], F32, tag="mm")
                    nc.tensor.matmul(
                        pmm[:],
                        otp_sb[:, g * 128:(g + 1) * 128],
                        wproj_sb[:, ntile * 512:(ntile + 1) * 512],
                        start=True, stop=True)
                    if use_scalar and ntile == 1:
                        nc.scalar.copy(ob[:, 512:1024], pmm[:])
                    else:
                        nc.vector.tensor_copy(
                            ob[:, ntile * 512:(ntile + 1) * 512], pmm[:])
                nc.sync.dma_start(
                    out=out[g * 128:(g + 1) * 128, :], in_=ob[:])

            def o_mms(po, es, b, kp, h, first, last):
                # po[65, 512] += [V|1]_kt.T @ exp(S^T)_kt for both k-tiles
                for j in range(2):
                    vt = b * NK + kp * 2 + j
                    nc.tensor.matmul(
                        po[:],
                        v_sb[:, vt, h, :],
                        es[:, j * 512:(j + 1) * 512],
                        start=(first and j == 0),
                        stop=(last and j == 1))

            # One attention q-block: S^T tiles -> exp -> O^T accumulation.
            # extras[kp] are unrelated PE work units woven into the ACT-wait
            # slots so the PE never idles (HAM stays at full clock).
            def emit_s2(b, h, qb, pre=(), extras=None):
                for u in pre:
                    u()
                hp = slice(h * 64, (h + 1) * 64)
                q0 = b * N + qb * QB
                po = ps.tile([128, 512], F32, tag="o")
                pending = []   # (es, kp), O-mms lag 2 iterations behind exp
                for kp in range(NK // 2):
                    pst = ps.tile([128, 1024], F32, tag="s")
                    for j in range(2):
                        k0 = b * N + (kp * 2 + j) * 128
                        nc.tensor.matmul(
                            pst[:, j * 512:(j + 1) * 512],
                            qk_sb[hp, 1, k0:k0 + 128],
                            qk_sb[hp, 0, q0:q0 + QB],
                            start=True, stop=True)
                    es = work.tile([128, 1024], BF, tag="es")
                    nc.scalar.activation(
                        es[:], pst[:], mybir.ActivationFunctionType.Exp)
                    if len(pending) >= 2:
                        pes, pkp = pending.pop(0)
                        o_mms(po, pes, b, pkp, h,
                              first=(pkp == 0), last=False)
                    if extras:
                        for u in extras.get(kp, ()):
                            u()
                    pending.append((es, kp))
                for pes, pkp in pending:
                    o_mms(po, pes, b, pkp, h,
                          first=(pkp == 0), last=(pkp == NK // 2 - 1))
                # normalize O^T rows by 1/l: fast reciprocal of row 64,
                # partition-broadcast, one elementwise multiply
                lrow = work.tile([1, 512], F32, tag="lrow")
                nc.vector.tensor_copy(lrow[:], po[64:65, :])
                linv = work.tile([1, 512], F32, tag="linv")
                nc.vector.reciprocal_approx_fast(linv[:], lrow[:])
                lb = work.tile([64, 512], F32, tag="lb")
                nc.gpsimd.partition_broadcast(lb[:], linv[:])
                if h == 0:
                    nc.vector.tensor_mul(
                        otp_sb[0:64, q0:q0 + QB], po[0:64, :], lb[:])
                else:
                    nc.vector.tensor_mul(
                        ot1_sb[:, q0:q0 + QB], po[0:64, :], lb[:])
                    # repack h1 rows into partitions 64:128 of the packed
                    # O^T tile (cross-partition SBUF->SBUF DMA, gpsimd
                    # queue to keep the sync sequencer free for out-DMAs)
                    nc.gpsimd.dma_start(
                        out=otp_sb[64:128, q0:q0 + QB],
                        in_=ot1_sb[:, q0:q0 + QB])

            # -- schedule ---------------------------------------------------
            def U(f, *a):
                return lambda: f(*a)

            def drain(q, kps=range(NK // 2), per=1):
                ex = {}
                for kp in kps:
                    ex[kp] = [q.popleft() for _ in range(min(per, len(q)))]
                return ex

            from collections import deque

            # minimal prefix for (b0, h0, qb0)
            emit_qk(1, 0)
            emit_qk(0, 0)
            emit_vt(0)
            emit_vtr(0)
            emit_vtr(1)
            # remaining batch-0 stage-1 units woven into qb0 just ahead of
            # their consumers (S needs KT chunk kp//2, O(kp) needs v tiles
            # 2kp..2kp+1 one iteration later)
            ex0 = {0: [U(emit_vtr, 2), U(emit_vtr, 3)],
                   1: [U(emit_qk, 1, 1), U(emit_vt, 1),
                       U(emit_vtr, 4), U(emit_vtr, 5)],
                   2: [U(emit_vtr, 6), U(emit_vtr, 7)],
                   3: [U(emit_qk, 1, 2), U(emit_vt, 2),
                       U(emit_vtr, 8), U(emit_vtr, 9)],
                   4: [U(emit_vtr, 10), U(emit_vtr, 11)],
                   5: [U(emit_qk, 1, 3), U(emit_vt, 3),
                       U(emit_vtr, 12), U(emit_vtr, 13)],
                   6: [U(emit_vtr, 14), U(emit_vtr, 15)]}
            emit_s2(0, 0, 0, extras=ex0)

            # batch-1 stage-1 spread across the rest of batch 0's attention
            qA = deque()
            for nt in range(4, 8):
                qA.append(U(emit_qk, 1, nt))
                qA.append(U(emit_qk, 0, nt))
                qA.append(U(emit_vt, nt))
                for t in range(4 * nt, 4 * nt + 4):
                    qA.append(U(emit_vtr, t))
            emit_s2(0, 0, 1, pre=[U(emit_qk, 0, 1)],
                    extras=drain(qA, kps=(0, 2, 4, 6)))
            emit_s2(0, 0, 2, pre=[U(emit_qk, 0, 2)],
                    extras=drain(qA, kps=(0, 2, 4, 6)))
            emit_s2(0, 0, 3, pre=[U(emit_qk, 0, 3)],
                    extras=drain(qA, kps=(0, 2, 4, 6)))
            for qb in range(NQB):
                emit_s2(0, 1, qb, extras=drain(qA, kps=(0, 2, 4, 6)))
            rest = list(qA)
            qA.clear()

            # batch-0 projection spread thinly across ALL of batch 1 so even
            # the final q-blocks keep PE filler work during ACT waits
            qB = deque(U(emit_proj, g) for g in range(NK))
            emit_s2(1, 0, 0, pre=rest, extras=drain(qB, kps=(1, 5)))
            for qb in range(1, NQB):
                emit_s2(1, 0, qb, extras=drain(qB, kps=(1, 5)))
            # batch-1 projection for q-blocks already finished weaves into
            # (b1, h1)'s later q-blocks; only the last q-block's tiles remain
            # for the tail
            emit_s2(1, 1, 0, extras=drain(qB, kps=(1, 5)))
            qC = deque()
            for qb in range(1, NQB):
                qC.extend(U(emit_proj, NK + (qb - 1) * 4 + i) for i in range(4))
                ex = drain(qB, kps=(1, 5))
                for kp, us in drain(qC, kps=(3, 7), per=2).items():
                    ex.setdefault(kp, []).extend(us)
                emit_s2(1, 1, qb, extras=ex)
            for u in qB:
                u()
            for u in qC:
                u()
            for i, g in enumerate(range(2 * NK - 4, 2 * NK)):
                emit_proj(g, use_scalar=(i % 2 == 0))
    nc.compile()
    return nc


def make_in_maps(x, w_qkv, w_proj):
    bf = ml_dtypes.bfloat16
    x2 = x.reshape(T, C)
    xT_np = np.ascontiguousarray(x2.T).astype(bf)
    in_maps = []
    for c in range(NCORES):
        s = c * 128
        wq = w_qkv[:, s:s + 128] * SCALE
        wk = w_qkv[:, C + s:C + s + 128]
        wqk_np = np.ascontiguousarray(
            np.concatenate([wq, wk], axis=1)).astype(bf)
        wv_np = np.ascontiguousarray(
            w_qkv[:, 2 * C + s:2 * C + s + 128]).astype(bf)
        wproj_np = np.ascontiguousarray(w_proj[s:s + 128, :]).astype(bf)
        in_maps.append({"xT": xT_np, "wqk": wqk_np, "wv": wv_np,
                        "wproj": wproj_np})
    return in_maps


def kernel(x, w_qkv, w_proj, b_proj):
    x = np.asarray(x, dtype=np.float32)
    w_qkv = np.asarray(w_qkv, dtype=np.float32)
    w_proj = np.asarray(w_proj, dtype=np.float32)
    b_proj = np.asarray(b_proj, dtype=np.float32)

    if "nc" not in _NC_CACHE:
        _NC_CACHE["nc"] = build()
    nc = _NC_CACHE["nc"]

    in_maps = make_in_maps(x, w_qkv, w_proj)
    res = run_bass_kernel_spmd(nc, in_maps, list(range(NCORES)))
    acc = np.zeros((T, C), dtype=np.float32)
    for r in res.results:
        acc += r["out"]
    acc += b_proj[None, :]
    return acc.reshape(B, N, C)


# revision 36
# speedup vs baseline: 1.0922x; 1.0324x over previous
"""Multi-head attention forward (B=2, N=2048, C=1024, H=16) on 8 TRN2 NeuronCores.

Tensor-parallel over heads: core c owns heads {2c, 2c+1}. Each core computes
QKV projection for its heads, full attention for its 4 (batch, head)
instances, and a partial output projection against its 128 rows of w_proj.
The host sums the 8 partial projections and adds the bias (row-parallel TP;
the all-reduce is the host-side unshard).

Per-core layouts (all matmul inputs bf16, PSUM accumulation f32):
  xT    [1024, 4096]  x^T, channel-major (replicated)
  wqk   [1024, 256]   [Wq_h0|Wq_h1|Wk_h0|Wk_h1] columns, Wq pre-scaled by D^-0.5
  wv    [1024, 128]   [Wv_h0|Wv_h1]
  wproj [128, 1024]   rows 128c:128c+128 of w_proj
  out   [4096, 1024]  f32 partial projection output

Attention per (b, h): S^T tiles [k=128, q=512] = KT_chunk.T @ QT (d-major, no
transposes), exp on ACT directly from PSUM (scale D^-0.5 pre-folded into Wq,
no max-subtraction -- scores are N(0,1) bounded), then O^T accumulation
po[*,q] += [V|1|0pad].T @ exp(S^T): one N=512 matmul per key tile with the
128-wide padded V stationary; row 64 of po accumulates the softmax
denominator l for free. Normalize via fast-reciprocal + gpsimd
partition-broadcast + one DVE multiply; head-1 rows repacked to partitions
64:128 by an SBUF->SBUF DMA so the projection is a single K=128 GEMM.

Scheduling: all independent PE work (batch-1 stage 1, projections) is woven
into the attention loops' ACT-wait slots so the PE instruction stream never
idles -- sub-microsecond idle gaps repeated per iteration make the HAM
re-throttle the PE clock from 2.4 to 1.2 GHz. Exp->O matmuls are skewed two
iterations. DMA descriptor issue (~0.64us each per sequencer) is spread
across the sync/scalar/gpsimd queues, with the 16 first-matmul-critical
descriptors issued first.
"""

import numpy as np
import ml_dtypes

import concourse.bass as bass
import concourse.tile as tile
from concourse import bacc, mybir
from concourse.bass_utils import run_bass_kernel_spmd
from concourse.masks import make_identity

B, N, C = 2, 2048, 1024
H = 16
D = C // H          # 64
SCALE = D ** -0.5
NCORES = 8
T = B * N           # 4096 tokens
KT = C // 128       # 8 k-tiles over the C contraction
TOK_TILES = T // 128  # 32
NK = N // 128       # 16 key tiles per sequence
QB = 512            # q block width
NQB = N // QB       # 4
BF = mybir.dt.bfloat16
F32 = mybir.dt.float32

_NC_CACHE = {}


def build():
    nc = bacc.Bacc("TRN2", target_bir_lowering=False, debug=False,
                   num_devices=NCORES)
    xT = nc.dram_tensor("xT", [C, T], BF, kind="ExternalInput").ap()
    wqk = nc.dram_tensor("wqk", [C, 256], BF, kind="ExternalInput").ap()
    wv = nc.dram_tensor("wv", [C, 128], BF, kind="ExternalInput").ap()
    wproj = nc.dram_tensor("wproj", [128, C], BF, kind="ExternalInput").ap()
    out = nc.dram_tensor("out", [T, C], F32, kind="ExternalOutput").ap()

    with tile.TileContext(nc) as tc:
        with tc.tile_pool(name="const", bufs=1) as const, \
             tc.tile_pool(name="work", bufs=5) as work, \
             tc.tile_pool(name="ps", bufs=2, space="PSUM") as ps:

            xt_sb = const.tile([128, KT, T], BF, tag="xt")
            wqk_sb = const.tile([128, KT, 256], BF, tag="wqk")
            wv_sb = const.tile([128, KT, 128], BF, tag="wv")
            wproj_sb = const.tile([128, C], BF, tag="wproj")
            qk_sb = const.tile([128, 2, T], BF, tag="qk")   # [qchan|kchan, token]
            vt_sb = const.tile([128, T], BF, tag="vt")      # V^T [vchan, token]
            v_sb = const.tile([128, TOK_TILES, 2, 128], BF, tag="v")  # per head [V|1|0pad]
            # normalized O^T [dchan, token]; h0 rows 0:64 written by DVE,
            # h1 rows 64:128 filled by SBUF->SBUF DMA repack from ot1_sb
            otp_sb = const.tile([128, T], BF, tag="otp")
            ot1_sb = const.tile([64, T], BF, tag="ot1")
            ident = const.tile([128, 128], BF, tag="ident")

            make_identity(nc, ident[:])

            # Spread DMA descriptor issue across engine queues -- a single
            # sequencer issues one descriptor per ~0.64us, which would gate
            # startup. wqk on sync (first QK needs it), wv on vector,
            # wproj on gpsimd; x^T chunks alternate sync/vector, nt-major
            # so stage-1 can start after the first 8 chunks.
            # The first QK unit needs all 8 wqk chunks + the 8 nt0 x^T
            # chunks -- issue those 16 first, split across the two HWDGE
            # sequencers, before anything else.
            def xt_dma(eng, nt, kt):
                eng.dma_start(
                    out=xt_sb[:, kt, nt * 512:(nt + 1) * 512],
                    in_=xT[kt * 128:(kt + 1) * 128,
                           nt * 512:(nt + 1) * 512])

            for kt in range(KT):
                eng = nc.sync if kt % 2 == 0 else nc.scalar
                eng.dma_start(out=wqk_sb[:, kt, :],
                              in_=wqk[kt * 128:(kt + 1) * 128, :])
                xt_dma(nc.sync if kt % 2 == 1 else nc.scalar, 0, kt)
            for kt in range(KT):
                xt_dma(nc.sync if kt % 2 == 0 else nc.scalar, 1, kt)
                nc.gpsimd.dma_start(out=wv_sb[:, kt, :],
                                    in_=wv[kt * 128:(kt + 1) * 128, :])
            nc.gpsimd.dma_start(out=wproj_sb[:], in_=wproj[:, :])
            for nt in range(2, T // 512):
                for kt in range(KT):
                    xt_dma(nc.sync if kt % 2 == 0 else nc.gpsimd, nt, kt)

            nc.vector.memset(v_sb[:, :, :, 64:65], 1.0)

            # -- work units -------------------------------------------------
            def emit_qk(mt, nt):
                pmm = ps.tile([128, 512], F32, tag="mm")
                for kt in range(KT):
                    nc.tensor.matmul(
                        pmm[:],
                        wqk_sb[:, kt, mt * 128:(mt + 1) * 128],
                        xt_sb[:, kt, nt * 512:(nt + 1) * 512],
                        start=(kt == 0), stop=(kt == KT - 1))
                nc.vector.tensor_copy(
                    qk_sb[:, mt, nt * 512:(nt + 1) * 512], pmm[:])

            def emit_vt(nt):
                # V^T chunk [128 vchan, 512 tok] with wv stationary
                pv = ps.tile([128, 512], F32, tag="mm")
                for kt in range(KT):
                    nc.tensor.matmul(
                        pv[:],
                        wv_sb[:, kt, :],
                        xt_sb[:, kt, nt * 512:(nt + 1) * 512],
                        start=(kt == 0), stop=(kt == KT - 1))
                nc.vector.tensor_copy(
                    vt_sb[:, nt * 512:(nt + 1) * 512], pv[:])

            def emit_vtr(t):
                # PE-transpose V^T tile back to token-major [128 tok, 128 d]
                pt = ps.tile([128, 128], BF, tag="mm")
                nc.tensor.transpose(
                    pt[:], vt_sb[:, t * 128:(t + 1) * 128], ident[:])
                nc.vector.tensor_copy(v_sb[:, t, 0, 0:64], pt[:, 0:64])
                nc.vector.tensor_copy(v_sb[:, t, 1, 0:64], pt[:, 64:128])
                # pad cols feed only po rows 65:127 (never read); small
                # per-tile memset keeps the big strided fill off the
                # startup-critical DVE stream
                nc.vector.memset(v_sb[:, t, :, 65:128], 0.0)

            def emit_proj(g, use_scalar=False):
                # out_tile = O^T_packed.T @ wproj (K=128, both heads)
                ob = work.tile([128, C], F32, tag="outstage")
                for ntile in range(2):
                    pmm = ps.tile([128, 512], F32, tag="mm")
                    nc.tensor.matmul(
                        pmm[:],
                        otp_sb[:, g * 128:(g + 1) * 128],
                        wproj_sb[:, ntile * 512:(ntile + 1) * 512],
                        start=True, stop=True)
                    if use_scalar and ntile == 1:
                        nc.scalar.copy(ob[:, 512:1024], pmm[:])
                    else:
                        nc.vector.tensor_copy(
                            ob[:, ntile * 512:(ntile + 1) * 512], pmm[:])
                nc.sync.dma_start(
                    out=out[g * 128:(g + 1) * 128, :], in_=ob[:])

            def o_mms(po, es, b, kp, h, first, last):
                # po[65, 512] += [V|1]_kt.T @ exp(S^T)_kt for both k-tiles
                for j in range(2):
                    vt = b * NK + kp * 2 + j
                    nc.tensor.matmul(
                        po[:],
                        v_sb[:, vt, h, :],
                        es[:, j * 512:(j + 1) * 512],
                        start=(first and j == 0),
                        stop=(last and j == 1))

            # One attention q-block: S^T tiles -> exp -> O^T accumulation.
            # extras[kp] are unrelated PE work units woven into the ACT-wait
            # slots so the PE never idles (HAM stays at full clock).
            def emit_s2(b, h, qb, pre=(), extras=None):
                for u in pre:
                    u()
                hp = slice(h * 64, (h + 1) * 64)
                q0 = b * N + qb * QB
                po = ps.tile([128, 512], F32, tag="o")
                pending = []   # (es, kp), O-mms lag 2 iterations behind exp
                for kp in range(NK // 2):
                    pst = ps.tile([128, 1024], F32, tag="s")
                    for j in range(2):
                        k0 = b * N + (kp * 2 + j) * 128
                        nc.tensor.matmul(
                            pst[:, j * 512:(j + 1) * 512],
                            qk_sb[hp, 1, k0:k0 + 128],
                            qk_sb[hp, 0, q0:q0 + QB],
                            start=True, stop=True)
                    es = work.tile([128, 1024], BF, tag="es")
                    nc.scalar.activation(
                        es[:], pst[:], mybir.ActivationFunctionType.Exp)
                    if len(pending) >= 3:
                        pes, pkp = pending.pop(0)
                        o_mms(po, pes, b, pkp, h,
                              first=(pkp == 0), last=False)
                    if extras:
                        for u in extras.get(kp, ()):
                            u()
                    pending.append((es, kp))
                for pes, pkp in pending:
                    o_mms(po, pes, b, pkp, h,
                          first=(pkp == 0), last=(pkp == NK // 2 - 1))
                # normalize O^T rows by 1/l: fast reciprocal of row 64,
                # partition-broadcast, one elementwise multiply
                lrow = work.tile([1, 512], F32, tag="lrow")
                nc.vector.tensor_copy(lrow[:], po[64:65, :])
                linv = work.tile([1, 512], F32, tag="linv")
                nc.vector.reciprocal_approx_fast(linv[:], lrow[:])
                lb = work.tile([64, 512], F32, tag="lb")
                nc.gpsimd.partition_broadcast(lb[:], linv[:])
                if h == 0:
                    nc.vector.tensor_mul(
                        otp_sb[0:64, q0:q0 + QB], po[0:64, :], lb[:])
                else:
                    nc.vector.tensor_mul(
                        ot1_sb[:, q0:q0 + QB], po[0:64, :], lb[:])
                    # repack h1 rows into partitions 64:128 of the packed
                    # O^T tile (cross-partition SBUF->SBUF DMA, gpsimd
                    # queue to keep the sync sequencer free for out-DMAs)
                    nc.gpsimd.dma_start(
                        out=otp_sb[64:128, q0:q0 + QB],
                        in_=ot1_sb[:, q0:q0 + QB])

            # -- schedule ---------------------------------------------------
            def U(f, *a):
                return lambda: f(*a)

            def drain(q, kps=range(NK // 2), per=1):
                ex = {}
                for kp in kps:
                    ex[kp] = [q.popleft() for _ in range(min(per, len(q)))]
                return ex

            from collections import deque

            # minimal prefix for (b0, h0, qb0)
            emit_qk(1, 0)
            emit_qk(0, 0)
            emit_vt(0)
            emit_vtr(0)
            emit_vtr(1)
            # remaining batch-0 stage-1 units woven into qb0 just ahead of
            # their consumers (S needs KT chunk kp//2, O(kp) needs v tiles
            # 2kp..2kp+1 one iteration later)
            ex0 = {0: [U(emit_vtr, 2), U(emit_vtr, 3)],
                   1: [U(emit_qk, 1, 1), U(emit_vt, 1),
                       U(emit_vtr, 4), U(emit_vtr, 5)],
                   2: [U(emit_vtr, 6), U(emit_vtr, 7)],
                   3: [U(emit_qk, 1, 2), U(emit_vt, 2),
                       U(emit_vtr, 8), U(emit_vtr, 9)],
                   4: [U(emit_vtr, 10), U(emit_vtr, 11)],
                   5: [U(emit_qk, 1, 3), U(emit_vt, 3),
                       U(emit_vtr, 12), U(emit_vtr, 13)],
                   6: [U(emit_vtr, 14), U(emit_vtr, 15)]}
            emit_s2(0, 0, 0, extras=ex0)

            # batch-1 stage-1 spread across the rest of batch 0's attention
            qA = deque()
            for nt in range(4, 8):
                qA.append(U(emit_qk, 1, nt))
                qA.append(U(emit_qk, 0, nt))
                qA.append(U(emit_vt, nt))
                for t in range(4 * nt, 4 * nt + 4):
                    qA.append(U(emit_vtr, t))
            emit_s2(0, 0, 1, pre=[U(emit_qk, 0, 1)],
                    extras=drain(qA, kps=(0, 2, 4, 6)))
            emit_s2(0, 0, 2, pre=[U(emit_qk, 0, 2)],
                    extras=drain(qA, kps=(0, 2, 4, 6)))
            emit_s2(0, 0, 3, pre=[U(emit_qk, 0, 3)],
                    extras=drain(qA, kps=(0, 2, 4, 6)))
            for qb in range(NQB):
                emit_s2(0, 1, qb, extras=drain(qA, kps=(0, 2, 4, 6)))
            rest = list(qA)
            qA.clear()

            # batch-0 projection spread thinly across ALL of batch 1 so even
            # the final q-blocks keep PE filler work during ACT waits
            qB = deque(U(emit_proj, g) for g in range(NK))
            emit_s2(1, 0, 0, pre=rest, extras=drain(qB, kps=(1, 5)))
            for qb in range(1, NQB):
                emit_s2(1, 0, qb, extras=drain(qB, kps=(1, 5)))
            # batch-1 projection for q-blocks already finished weaves into
            # (b1, h1)'s later q-blocks; only the last q-block's tiles remain
            # for the tail
            emit_s2(1, 1, 0, extras=drain(qB, kps=(1, 5)))
            qC = deque()
            for qb in range(1, NQB):
                qC.extend(U(emit_proj, NK + (qb - 1) * 4 + i) for i in range(4))
                ex = drain(qB, kps=(1, 5))
                for kp, us in drain(qC, kps=(3, 7), per=2).items():
                    ex.setdefault(kp, []).extend(us)
                emit_s2(1, 1, qb, extras=ex)
            for u in qB:
                u()
            for u in qC:
                u()
            for i, g in enumerate(range(2 * NK - 4, 2 * NK)):
                emit_proj(g, use_scalar=(i % 2 == 0))
    nc.compile()
    return nc


def make_in_maps(x, w_qkv, w_proj):
    bf = ml_dtypes.bfloat16
    x2 = x.reshape(T, C)
    xT_np = np.ascontiguousarray(x2.T).astype(bf)
    in_maps = []
    for c in range(NCORES):
        s = c * 128
        wq = w_qkv[:, s:s + 128] * SCALE
        wk = w_qkv[:, C + s:C + s + 128]
        wqk_np = np.ascontiguousarray(
            np.concatenate([wq, wk], axis=1)).astype(bf)
        wv_np = np.ascontiguousarray(
            w_qkv[:, 2 * C + s:2 * C + s + 128]).astype(bf)
        wproj_np = np.ascontiguousarray(w_proj[s:s + 128, :]).astype(bf)
        in_maps.append({"xT": xT_np, "wqk": wqk_np, "wv": wv_np,
                        "wproj": wproj_np})
    return in_maps


def kernel(x, w_qkv, w_proj, b_proj):
    x = np.asarray(x, dtype=np.float32)
    w_qkv = np.asarray(w_qkv, dtype=np.float32)
    w_proj = np.asarray(w_proj, dtype=np.float32)
    b_proj = np.asarray(b_proj, dtype=np.float32)

    if "nc" not in _NC_CACHE:
        _NC_CACHE["nc"] = build()
    nc = _NC_CACHE["nc"]

    in_maps = make_in_maps(x, w_qkv, w_proj)
    res = run_bass_kernel_spmd(nc, in_maps, list(range(NCORES)))
    acc = np.zeros((T, C), dtype=np.float32)
    for r in res.results:
        acc += r["out"]
    acc += b_proj[None, :]
    return acc.reshape(B, N, C)


# revision 38
# speedup vs baseline: 1.0996x; 1.0067x over previous
"""Multi-head attention forward (B=2, N=2048, C=1024, H=16) on 8 TRN2 NeuronCores.

Tensor-parallel over heads: core c owns heads {2c, 2c+1}. Each core computes
QKV projection for its heads, full attention for its 4 (batch, head)
instances, and a partial output projection against its 128 rows of w_proj.
The host sums the 8 partial projections and adds the bias (row-parallel TP;
the all-reduce is the host-side unshard).

Per-core layouts (all matmul inputs bf16, PSUM accumulation f32):
  xT    [1024, 4096]  x^T, channel-major (replicated)
  wqk   [1024, 256]   [Wq_h0|Wq_h1|Wk_h0|Wk_h1] columns, Wq pre-scaled by D^-0.5
  wv    [1024, 128]   [Wv_h0|Wv_h1]
  wproj [128, 1024]   rows 128c:128c+128 of w_proj
  out   [4096, 1024]  f32 partial projection output

Attention per (b, h): S^T tiles [k=128, q=512] = KT_chunk.T @ QT (d-major, no
transposes), exp on ACT directly from PSUM (scale D^-0.5 pre-folded into Wq,
no max-subtraction -- scores are N(0,1) bounded), then O^T accumulation
po[*,q] += [V|1|0pad].T @ exp(S^T): one N=512 matmul per key tile with the
128-wide padded V stationary; row 64 of po accumulates the softmax
denominator l for free. Normalize via fast-reciprocal + gpsimd
partition-broadcast + one DVE multiply; head-1 rows repacked to partitions
64:128 by an SBUF->SBUF DMA so the projection is a single K=128 GEMM.

Scheduling: all independent PE work (batch-1 stage 1, projections) is woven
into the attention loops' ACT-wait slots so the PE instruction stream never
idles -- sub-microsecond idle gaps repeated per iteration make the HAM
re-throttle the PE clock from 2.4 to 1.2 GHz. Exp->O matmuls are skewed
three iterations. DMA descriptor issue (~0.64us each per sequencer) is spread
across the sync/scalar/gpsimd queues, with the 16 first-matmul-critical
descriptors issued first.
"""

import numpy as np
import ml_dtypes

import concourse.bass as bass
import concourse.tile as tile
from concourse import bacc, mybir
from concourse.bass_utils import run_bass_kernel_spmd
from concourse.masks import make_identity

B, N, C = 2, 2048, 1024
H = 16
D = C // H          # 64
SCALE = D ** -0.5
NCORES = 8
T = B * N           # 4096 tokens
KT = C // 128       # 8 k-tiles over the C contraction
TOK_TILES = T // 128  # 32
NK = N // 128       # 16 key tiles per sequence
QB = 512            # q block width
NQB = N // QB       # 4
BF = mybir.dt.bfloat16
F32 = mybir.dt.float32

_NC_CACHE = {}


def build():
    nc = bacc.Bacc("TRN2", target_bir_lowering=False, debug=False,
                   num_devices=NCORES)
    xT = nc.dram_tensor("xT", [C, T], BF, kind="ExternalInput").ap()
    wqk = nc.dram_tensor("wqk", [C, 256], BF, kind="ExternalInput").ap()
    wv = nc.dram_tensor("wv", [C, 128], BF, kind="ExternalInput").ap()
    wproj = nc.dram_tensor("wproj", [128, C], BF, kind="ExternalInput").ap()
    out = nc.dram_tensor("out", [T, C], F32, kind="ExternalOutput").ap()

    with tile.TileContext(nc) as tc:
        with tc.tile_pool(name="const", bufs=1) as const, \
             tc.tile_pool(name="work", bufs=5) as work, \
             tc.tile_pool(name="ps", bufs=2, space="PSUM") as ps:

            xt_sb = const.tile([128, KT, T], BF, tag="xt")
            wqk_sb = const.tile([128, KT, 256], BF, tag="wqk")
            wv_sb = const.tile([128, KT, 128], BF, tag="wv")
            wproj_sb = const.tile([128, C], BF, tag="wproj")
            qk_sb = const.tile([128, 2, T], BF, tag="qk")   # [qchan|kchan, token]
            vt_sb = const.tile([128, T], BF, tag="vt")      # V^T [vchan, token]
            v_sb = const.tile([128, TOK_TILES, 2, 128], BF, tag="v")  # per head [V|1|0pad]
            # normalized O^T [dchan, token]; h0 rows 0:64 written by DVE,
            # h1 rows 64:128 filled by SBUF->SBUF DMA repack from ot1_sb
            otp_sb = const.tile([128, T], BF, tag="otp")
            ot1_sb = const.tile([64, T], BF, tag="ot1")
            ident = const.tile([128, 128], BF, tag="ident")

            make_identity(nc, ident[:])

            # Spread DMA descriptor issue across engine queues -- a single
            # sequencer issues one descriptor per ~0.64us, which would gate
            # startup. wqk on sync (first QK needs it), wv on vector,
            # wproj on gpsimd; x^T chunks alternate sync/vector, nt-major
            # so stage-1 can start after the first 8 chunks.
            # The first QK unit needs all 8 wqk chunks + the 8 nt0 x^T
            # chunks -- issue those 16 first, split across the two HWDGE
            # sequencers, before anything else.
            def xt_dma(eng, nt, kt):
                eng.dma_start(
                    out=xt_sb[:, kt, nt * 512:(nt + 1) * 512],
                    in_=xT[kt * 128:(kt + 1) * 128,
                           nt * 512:(nt + 1) * 512])

            for kt in range(KT):
                eng = nc.sync if kt % 2 == 0 else nc.scalar
                eng.dma_start(out=wqk_sb[:, kt, :],
                              in_=wqk[kt * 128:(kt + 1) * 128, :])
                xt_dma(nc.sync if kt % 2 == 1 else nc.scalar, 0, kt)
            for kt in range(KT):
                xt_dma(nc.sync if kt % 2 == 0 else nc.scalar, 1, kt)
                nc.gpsimd.dma_start(out=wv_sb[:, kt, :],
                                    in_=wv[kt * 128:(kt + 1) * 128, :])
            nc.gpsimd.dma_start(out=wproj_sb[:], in_=wproj[:, :])
            for nt in range(2, T // 512):
                for kt in range(KT):
                    xt_dma(nc.sync if kt % 2 == 0 else nc.gpsimd, nt, kt)

            nc.vector.memset(v_sb[:, :, :, 64:65], 1.0)

            # -- work units -------------------------------------------------
            def emit_qk(mt, nt):
                pmm = ps.tile([128, 512], F32, tag="mm")
                for kt in range(KT):
                    nc.tensor.matmul(
                        pmm[:],
                        wqk_sb[:, kt, mt * 128:(mt + 1) * 128],
                        xt_sb[:, kt, nt * 512:(nt + 1) * 512],
                        start=(kt == 0), stop=(kt == KT - 1))
                nc.vector.tensor_copy(
                    qk_sb[:, mt, nt * 512:(nt + 1) * 512], pmm[:])

            def emit_vt(nt):
                # V^T chunk [128 vchan, 512 tok] with wv stationary
                pv = ps.tile([128, 512], F32, tag="mm")
                for kt in range(KT):
                    nc.tensor.matmul(
                        pv[:],
                        wv_sb[:, kt, :],
                        xt_sb[:, kt, nt * 512:(nt + 1) * 512],
                        start=(kt == 0), stop=(kt == KT - 1))
                nc.vector.tensor_copy(
                    vt_sb[:, nt * 512:(nt + 1) * 512], pv[:])

            def emit_vtr(t):
                # PE-transpose V^T tile back to token-major [128 tok, 128 d]
                pt = ps.tile([128, 128], BF, tag="mm")
                nc.tensor.transpose(
                    pt[:], vt_sb[:, t * 128:(t + 1) * 128], ident[:])
                nc.vector.tensor_copy(v_sb[:, t, 0, 0:64], pt[:, 0:64])
                nc.vector.tensor_copy(v_sb[:, t, 1, 0:64], pt[:, 64:128])
                # pad cols feed only po rows 65:127 (never read); small
                # per-tile memset keeps the big strided fill off the
                # startup-critical DVE stream
                nc.vector.memset(v_sb[:, t, :, 65:128], 0.0)

            def emit_proj(g, use_scalar=False):
                # out_tile = O^T_packed.T @ wproj (K=128, both heads)
                ob = work.tile([128, C], F32, tag="outstage")
                for ntile in range(2):
                    pmm = ps.tile([128, 512], F32, tag="mm")
                    nc.tensor.matmul(
                        pmm[:],
                        otp_sb[:, g * 128:(g + 1) * 128],
                        wproj_sb[:, ntile * 512:(ntile + 1) * 512],
                        start=True, stop=True)
                    if use_scalar and ntile == 1:
                        nc.scalar.copy(ob[:, 512:1024], pmm[:])
                    else:
                        nc.vector.tensor_copy(
                            ob[:, ntile * 512:(ntile + 1) * 512], pmm[:])
                nc.sync.dma_start(
                    out=out[g * 128:(g + 1) * 128, :], in_=ob[:])

            def o_mms(po, es, b, kp, h, first, last):
                # po[65, 512] += [V|1]_kt.T @ exp(S^T)_kt for both k-tiles
                for j in range(2):
                    vt = b * NK + kp * 2 + j
                    nc.tensor.matmul(
                        po[:],
                        v_sb[:, vt, h, :],
                        es[:, j * 512:(j + 1) * 512],
                        start=(first and j == 0),
                        stop=(last and j == 1))

            # One attention q-block: S^T tiles -> exp -> O^T accumulation.
            # extras[kp] are unrelated PE work units woven into the ACT-wait
            # slots so the PE never idles (HAM stays at full clock).
            def emit_s2(b, h, qb, pre=(), extras=None):
                for u in pre:
                    u()
                hp = slice(h * 64, (h + 1) * 64)
                q0 = b * N + qb * QB
                po = ps.tile([128, 512], F32, tag="o")
                pending = []   # (es, kp), O-mms lag 3 iterations behind exp
                for kp in range(NK // 2):
                    # O-pair + extras run BEFORE this iteration's S-pair so
                    # the S matmuls' psum-slot waits (on exp reads) have
                    # cleared by the time the engine reaches them -- an
                    # uncleared wait blocks the LDWEIGHTS prefetch and costs
                    # ~95ns per matmul
                    if len(pending) >= 3:
                        pes, pkp = pending.pop(0)
                        o_mms(po, pes, b, pkp, h,
                              first=(pkp == 0), last=False)
                    if extras:
                        for u in extras.get(kp, ()):
                            u()
                    pst = ps.tile([128, 1024], F32, tag="s")
                    for j in range(2):
                        k0 = b * N + (kp * 2 + j) * 128
                        nc.tensor.matmul(
                            pst[:, j * 512:(j + 1) * 512],
                            qk_sb[hp, 1, k0:k0 + 128],
                            qk_sb[hp, 0, q0:q0 + QB],
                            start=True, stop=True)
                    es = work.tile([128, 1024], BF, tag="es")
                    nc.scalar.activation(
                        es[:], pst[:], mybir.ActivationFunctionType.Exp)
                    pending.append((es, kp))
                for pes, pkp in pending:
                    o_mms(po, pes, b, pkp, h,
                          first=(pkp == 0), last=(pkp == NK // 2 - 1))
                # normalize O^T rows by 1/l: fast reciprocal of row 64,
                # partition-broadcast, one elementwise multiply
                lrow = work.tile([1, 512], F32, tag="lrow")
                nc.vector.tensor_copy(lrow[:], po[64:65, :])
                linv = work.tile([1, 512], F32, tag="linv")
                nc.vector.reciprocal_approx_fast(linv[:], lrow[:])
                lb = work.tile([64, 512], F32, tag="lb")
                nc.gpsimd.partition_broadcast(lb[:], linv[:])
                if h == 0:
                    nc.vector.tensor_mul(
                        otp_sb[0:64, q0:q0 + QB], po[0:64, :], lb[:])
                else:
                    nc.vector.tensor_mul(
                        ot1_sb[:, q0:q0 + QB], po[0:64, :], lb[:])
                    # repack h1 rows into partitions 64:128 of the packed
                    # O^T tile (cross-partition SBUF->SBUF DMA, gpsimd
                    # queue to keep the sync sequencer free for out-DMAs)
                    nc.gpsimd.dma_start(
                        out=otp_sb[64:128, q0:q0 + QB],
                        in_=ot1_sb[:, q0:q0 + QB])

            # -- schedule ---------------------------------------------------
            def U(f, *a):
                return lambda: f(*a)

            def drain(q, kps=range(NK // 2), per=1):
                ex = {}
                for kp in kps:
                    ex[kp] = [q.popleft() for _ in range(min(per, len(q)))]
                return ex

            from collections import deque

            # minimal prefix for (b0, h0, qb0)
            emit_qk(1, 0)
            emit_qk(0, 0)
            emit_vt(0)
            emit_vtr(0)
            emit_vtr(1)
            # remaining batch-0 stage-1 units woven into qb0 just ahead of
            # their consumers (S needs KT chunk kp//2, O(kp) needs v tiles
            # 2kp..2kp+1 one iteration later)
            ex0 = {0: [U(emit_vtr, 2), U(emit_vtr, 3)],
                   1: [U(emit_qk, 1, 1), U(emit_vt, 1),
                       U(emit_vtr, 4), U(emit_vtr, 5)],
                   2: [U(emit_vtr, 6), U(emit_vtr, 7)],
                   3: [U(emit_qk, 1, 2), U(emit_vt, 2),
                       U(emit_vtr, 8), U(emit_vtr, 9)],
                   4: [U(emit_vtr, 10), U(emit_vtr, 11)],
                   5: [U(emit_qk, 1, 3), U(emit_vt, 3),
                       U(emit_vtr, 12), U(emit_vtr, 13)],
                   6: [U(emit_vtr, 14), U(emit_vtr, 15)]}
            emit_s2(0, 0, 0, extras=ex0)

            # batch-1 stage-1 spread across the rest of batch 0's attention
            qA = deque()
            for nt in range(4, 8):
                qA.append(U(emit_qk, 1, nt))
                qA.append(U(emit_qk, 0, nt))
                qA.append(U(emit_vt, nt))
                for t in range(4 * nt, 4 * nt + 4):
                    qA.append(U(emit_vtr, t))
            emit_s2(0, 0, 1, pre=[U(emit_qk, 0, 1)],
                    extras=drain(qA, kps=(0, 2, 4, 6)))
            emit_s2(0, 0, 2, pre=[U(emit_qk, 0, 2)],
                    extras=drain(qA, kps=(0, 2, 4, 6)))
            emit_s2(0, 0, 3, pre=[U(emit_qk, 0, 3)],
                    extras=drain(qA, kps=(0, 2, 4, 6)))
            for qb in range(NQB):
                emit_s2(0, 1, qb, extras=drain(qA, kps=(0, 2, 4, 6)))
            rest = list(qA)
            qA.clear()

            # batch-0 projection spread thinly across ALL of batch 1 so even
            # the final q-blocks keep PE filler work during ACT waits
            qB = deque(U(emit_proj, g) for g in range(NK))
            emit_s2(1, 0, 0, pre=rest, extras=drain(qB, kps=(1, 5)))
            for qb in range(1, NQB):
                emit_s2(1, 0, qb, extras=drain(qB, kps=(1, 5)))
            # batch-1 projection for q-blocks already finished weaves into
            # (b1, h1)'s later q-blocks; only the last q-block's tiles remain
            # for the tail
            emit_s2(1, 1, 0, extras=drain(qB, kps=(1, 5)))
            qC = deque()
            for qb in range(1, NQB):
                qC.extend(U(emit_proj, NK + (qb - 1) * 4 + i) for i in range(4))
                ex = drain(qB, kps=(1, 5))
                for kp, us in drain(qC, kps=(3, 7), per=2).items():
                    ex.setdefault(kp, []).extend(us)
                emit_s2(1, 1, qb, extras=ex)
            for u in qB:
                u()
            for u in qC:
                u()
            for i, g in enumerate(range(2 * NK - 4, 2 * NK)):
                emit_proj(g, use_scalar=(i % 2 == 0))
    nc.compile()
    return nc


def make_in_maps(x, w_qkv, w_proj):
    bf = ml_dtypes.bfloat16
    x2 = x.reshape(T, C)
    xT_np = np.ascontiguousarray(x2.T).astype(bf)
    in_maps = []
    for c in range(NCORES):
        s = c * 128
        wq = w_qkv[:, s:s + 128] * SCALE
        wk = w_qkv[:, C + s:C + s + 128]
        wqk_np = np.ascontiguousarray(
            np.concatenate([wq, wk], axis=1)).astype(bf)
        wv_np = np.ascontiguousarray(
            w_qkv[:, 2 * C + s:2 * C + s + 128]).astype(bf)
        wproj_np = np.ascontiguousarray(w_proj[s:s + 128, :]).astype(bf)
        in_maps.append({"xT": xT_np, "wqk": wqk_np, "wv": wv_np,
                        "wproj": wproj_np})
    return in_maps


def kernel(x, w_qkv, w_proj, b_proj):
    x = np.asarray(x, dtype=np.float32)
    w_qkv = np.asarray(w_qkv, dtype=np.float32)
    w_proj = np.asarray(w_proj, dtype=np.float32)
    b_proj = np.asarray(b_proj, dtype=np.float32)

    if "nc" not in _NC_CACHE:
        _NC_CACHE["nc"] = build()
    nc = _NC_CACHE["nc"]

    in_maps = make_in_maps(x, w_qkv, w_proj)
    res = run_bass_kernel_spmd(nc, in_maps, list(range(NCORES)))
    acc = np.zeros((T, C), dtype=np.float32)
    for r in res.results:
        acc += r["out"]
    acc += b_proj[None, :]
    return acc.reshape(B, N, C)
